# revision 19
# baseline (speedup 1.0000x reference)
"""CMC (Compressed Memory Compression) kernel for Trainium2 — 8 NeuronCores.

Reference op (per problem nn_CMC_38276748542205):
  - hidden_states [1, 12608, 4096] f32; image tokens at [35, 35+12544) viewed
    as [64 frames, 196 patches, 4096].
  - Frames form 16 intervals of 4; I-frame at position 3 of each interval.
  - SAD(token, I-frame token at same patch) over dim; mask = SAD < 1.12*4096.
  - Masked tokens replaced by the interval's I-frame token.

Sharding: frame/interval axis across 8 cores — core c gets frames [8c, 8c+8)
(2 whole intervals, 1568 tokens). Text tokens (64 rows) pass through on host.

Device kernel (per core, SPMD) — sparse-scatter formulation:
  The replacement value for a masked token is bit-exactly the interval's
  I-frame token, and unmasked tokens are bit-exactly the input — so a full
  write-back (25.7 MB/core) is wasted HBM traffic.  Instead:
  - stream patch-major chunks as [128 patches, 2 frames, 4096] half-tiles
    (I-half f2+f3, P-half f0+f1) via strided 4 MB DMAs on the two HWDGE
    rings;
  - DVE: d_k = p3 - p_k; ACT: |d_k| in-place with per-2048-chunk
    accumulation -> SAD (chunked so fp32 summation error stays below the
    min |SAD-thr| margin of ~0.034 — the 2e-2 rel-err budget only allows
    ~2 flipped tokens, so the SAD math must stay exact-fp32);
  - DVE: m = (sad < thr) per-partition 0/1 into a [128, 12] mask tile
    (one column per interval/frame/chunk), stored once at the end (6 KB)
    for the host merge;
  - scatter index: idx = iota_col*m + BIG -> masked rows get their patch
    row, unmasked rows get BIG (OOB); gpsimd indirect DMA with
    bounds_check silently skips OOB rows, writing ONLY the ~26% replaced
    rows (the f3 tile) into that (iv,k,chunk)'s own output tensor.
    Disjoint write sets per tensor keep the tile scheduler from chaining
    the scatters on a conservative WAW hazard; indirect DMA APs must
    start at partition 0 (non-zero start wedges the device), so chunk B
    scatters a [0:96] window with rows [0:32) forced OOB.
  Device traffic: ~24 MB loads + ~5 MB scatter vs 51.4 MB baseline. Host
  merge: out starts as a copy of the input; only mask-selected rows are
  copied from the scatter outputs (everything else is already correct by
  identity).

DMA shape rules (measured on HW):
  - the 16 SDMA engines split a transfer's partition dim into gcd(P,16)
    groups -> P must be a multiple of 16;
  - even SBUF AXI ports serve partitions <64, odd ports >=64 -> full rate
    needs the window balanced across the 64-boundary (128 rows, or 64
    rows at [32:96]; 64 rows at [0:64) run ~half rate);
  - compute APs must start at partition 0 (32/96 allow <=32 rows, 64
    allows <=64).
Patch coverage: chunk A = patches 0-127 at [0:128]; chunk B = patches
128-191 at partitions [32:96] (compute on [0:96]; the never-loaded rows
[0:32) produce garbage masks that are ignored: their scatter indices are
forced OOB and the host decodes only [32:96)). Patches 192-195 (the %16
runt) are handled host-side in numpy.
"""

import functools
import os

import numpy as np

# ---- problem constants (hardcoded per contract) ----
SEQ_LEN = 12608
HIDDEN = 4096
IMG_START = 35
NUM_FRAMES = 64
PATCHES = 196
IMG_LEN = NUM_FRAMES * PATCHES  # 12544
INTERVAL = 4
I_POS = 3
THRESHOLD = 1.12 * HIDDEN  # 4587.52

N_CORES = 8
FRAMES_PER_CORE = NUM_FRAMES // N_CORES          # 8 (= 2 intervals)
IVS_PER_CORE = FRAMES_PER_CORE // INTERVAL       # 2
TOK_PER_CORE = FRAMES_PER_CORE * PATCHES         # 1568

SAD_CHUNK = 2048       # accumulation chunk for SAD numerical accuracy
BIG = 4096.0           # OOB scatter index (> PATCHES-1 -> skipped)
N_MASK_COLS = IVS_PER_CORE * 3 * 2  # 12: (interval, P-frame k, chunk A/B)

# chunk table: (partition lo, partition hi, patch offset pbase) with
# patch = pbase + partition for partitions in [lo, hi)
CHUNKS = ((0, 128, 0), (32, 96, 96))


def _kernel_body(tc, ys_aps, ym_ap, x_ap):
    import concourse.bass as bass
    from concourse import mybir

    nc = tc.nc
    AF = mybir.ActivationFunctionType
    OP = mybir.AluOpType
    f32 = mybir.dt.float32
    i32 = mybir.dt.int32
    sim_init = bool(os.environ.get("CMC_SIM_INIT"))

    xv = x_ap.rearrange("(f p) d -> p f d", f=FRAMES_PER_CORE, p=PATCHES)

    import contextlib

    with contextlib.ExitStack() as ctx:
        pI_pool = ctx.enter_context(tc.tile_pool(name="pI", bufs=3))
        pP_pool = ctx.enter_context(tc.tile_pool(name="pP", bufs=2))
        d_pool = ctx.enter_context(tc.tile_pool(name="d", bufs=2))
        small_pool = ctx.enter_context(tc.tile_pool(name="small", bufs=12))
        hold_pool = ctx.enter_context(tc.tile_pool(name="hold", bufs=1))

        n_sad_chunks = HIDDEN // SAD_CHUNK

        # one-time tiles: per-chunk scatter index bases
        # iota_c[:, cb] = p + pbase - BIG
        iota_i = hold_pool.tile([128, 1], i32, tag="iotai")
        nc.gpsimd.iota(iota_i[:, :], [[0, 1]], base=0, channel_multiplier=1)
        iota_f = hold_pool.tile([128, 1], f32, tag="iotaf")
        nc.vector.tensor_copy(iota_f[:, :], iota_i[:, :])
        iota_c = hold_pool.tile([128, 2], f32, tag="iotac")
        for cb, (lo, hi, pbase) in enumerate(CHUNKS):
            nc.vector.tensor_scalar(
                iota_c[:, cb : cb + 1],
                iota_f[:, :],
                float(pbase - BIG),
                None,
                op0=OP.add,
            )
        mask_sb = hold_pool.tile([128, N_MASK_COLS], f32, tag="mask")
        nc.vector.memset(mask_sb[:, :], 0.0)

        def compute_k(pt3, ptk, iv, cb, k):
            lo, hi, pbase = CHUNKS[cb]
            q1 = hi
            d_t = d_pool.tile([128, HIDDEN], f32)
            nc.vector.tensor_tensor(
                d_t[:q1, :], pt3[:q1, :], ptk[:q1, :], op=OP.subtract
            )
            sadp = small_pool.tile([128, n_sad_chunks], f32, tag="sadp")
            for h in range(n_sad_chunks):
                # |d| in place (out aliases in); only accum_out is consumed
                nc.scalar.activation(
                    d_t[:q1, bass.ts(h, SAD_CHUNK)],
                    d_t[:q1, bass.ts(h, SAD_CHUNK)],
                    AF.Abs,
                    accum_out=sadp[:q1, h : h + 1],
                )
            col = iv * 6 + k * 2 + cb
            m_col = mask_sb[:, col : col + 1]
            # fused: m = (sadp0 + sadp1) < thr — both scalars per-partition
            nc.vector.tensor_scalar(
                m_col[:q1, :],
                sadp[:q1, 0:1],
                sadp[:q1, 1:2],
                float(THRESHOLD),
                op0=OP.add,
                op1=OP.is_lt,
            )
            # scatter index: masked -> patch row, unmasked -> BIG (OOB)
            idx_f = small_pool.tile([128, 1], f32, tag="idxf")
            nc.vector.tensor_scalar(
                idx_f[:q1, :],
                iota_c[:q1, cb : cb + 1],
                m_col[:q1, 0:1],
                BIG,
                op0=OP.mult,
                op1=OP.add,
            )
            idx_i = small_pool.tile([128, 1], i32, tag="idxi")
            nc.vector.tensor_copy(idx_i[:q1, :], idx_f[:q1, :])
            if lo > 0:
                # never-loaded rows: force out of bounds
                nc.vector.memset(idx_i[0:lo, :], int(BIG))
            # each (iv, k, chunk) scatter targets its own tensor: disjoint
            # write sets, so the tile scheduler doesn't chain them on a
            # conservative WAW hazard (one shared target serializes all 12
            # scatters on each other's DMA-completion semaphores)
            nc.gpsimd.indirect_dma_start(
                out=ys_aps[col],
                out_offset=bass.IndirectOffsetOnAxis(
                    ap=idx_i[0:hi, 0:1], axis=0
                ),
                in_=pt3[0:hi, :],
                in_offset=None,
                bounds_check=PATCHES - 1,
                oob_is_err=False,
            )

        ld = [0]

        def load(dst, src):
            # alternate the two HWDGE rings so streams interleave
            eng = nc.sync if ld[0] % 2 == 0 else nc.scalar
            ld[0] += 1
            eng.dma_start(dst, src)

        first = True
        for iv in range(IVS_PER_CORE):
            f0 = iv * INTERVAL
            for cb, (lo, hi, pbase) in enumerate(CHUNKS):
                pw = slice(pbase + lo, pbase + hi)  # patch window in DRAM
                ptI = pI_pool.tile([128, 2, HIDDEN], f32, tag="ptI")
                ptP = pP_pool.tile([128, 2, HIDDEN], f32, tag="ptP")
                if sim_init and lo > 0:
                    nc.vector.memset(ptI[0:lo, :, :], 0.0)
                    nc.vector.memset(ptP[0:lo, :, :], 0.0)
                # 4 MB per dma_start: ring efficiency ~ bytes/(bytes +
                # ~2us fixed), so bigger transfers stream faster.  The very
                # first I-half instead goes as two 2 MB halves on BOTH
                # rings concurrently — nothing else is in flight yet, and
                # it pulls the first k=2 unit ~10 us earlier.
                if first:
                    load(ptI[lo:hi, 1:2, :], xv[pw, f0 + 3 : f0 + 4, :])
                    load(ptI[lo:hi, 0:1, :], xv[pw, f0 + 2 : f0 + 3, :])
                    first = False
                else:
                    load(ptI[lo:hi, :, :], xv[pw, f0 + 2 : f0 + 4, :])
                load(ptP[lo:hi, :, :], xv[pw, f0 : f0 + 2, :])
                for k in (2, 0, 1):  # f=3 (I-frame) never changes
                    ptk = ptI[:, 0, :] if k == 2 else ptP[:, k, :]
                    compute_k(ptI[:, 1, :], ptk, iv, cb, k)

        # single 6 KB mask store for the host merge
        nc.sync.dma_start(ym_ap, mask_sb[:, :])


@functools.cache
def _build_nc():
    import concourse.bacc as bacc
    import concourse.tile as tile
    from concourse import mybir

    nc = bacc.Bacc(
        "TRN2",
        target_bir_lowering=False,
        debug=False,
        enable_asserts=False,
        num_devices=N_CORES,
    )
    x = nc.dram_tensor(
        "x", [TOK_PER_CORE, HIDDEN], mybir.dt.float32, kind="ExternalInput"
    ).ap()
    ys = [
        nc.dram_tensor(
            f"ys{c}", [PATCHES, HIDDEN], mybir.dt.float32, kind="ExternalOutput"
        ).ap()
        for c in range(N_MASK_COLS)
    ]
    ym = nc.dram_tensor(
        "ym", [128, N_MASK_COLS], mybir.dt.float32, kind="ExternalOutput"
    ).ap()
    with tile.TileContext(nc) as tc:
        _kernel_body(tc, ys, ym, x)
    nc.compile()
    return nc


def _in_maps(hs: np.ndarray):
    img = hs[0, IMG_START : IMG_START + IMG_LEN]
    maps = []
    for c in range(N_CORES):
        xc = img[TOK_PER_CORE * c : TOK_PER_CORE * (c + 1)]
        maps.append({"x": np.ascontiguousarray(xc)})
    return maps


def _host_runt(img: np.ndarray) -> np.ndarray:
    """Blend for patches 192-195 (the %16 runt the device skips): numpy."""
    iv = img.reshape(NUM_FRAMES // INTERVAL, INTERVAL, PATCHES, HIDDEN)
    runt = iv[:, :, 192:PATCHES, :]
    itok = runt[:, I_POS : I_POS + 1]
    d = itok.astype(np.float64) - runt.astype(np.float64)
    mask = np.abs(d).sum(-1) < THRESHOLD
    return np.where(mask[..., None], itok, runt).astype(np.float32)


def kernel(hidden_states: np.ndarray) -> np.ndarray:
    from concourse.bass_utils import run_bass_kernel_spmd

    hs = np.asarray(hidden_states, dtype=np.float32)
    assert hs.shape == (1, SEQ_LEN, HIDDEN), hs.shape
    nc = _build_nc()
    res = run_bass_kernel_spmd(nc, _in_maps(hs), list(range(N_CORES)))
    out = hs.copy()
    img_flat = out[0, IMG_START : IMG_START + IMG_LEN]  # [12544, 4096] view
    for c in range(N_CORES):
        ym = np.asarray(res.results[c]["ym"])   # [128, 12] 0/1 mask columns
        for ivx in range(IVS_PER_CORE):
            for k in range(3):
                for cb, (lo, hi, pbase) in enumerate(CHUNKS):
                    col = ivx * 6 + k * 2 + cb
                    sel = np.nonzero(ym[lo:hi, col] > 0.5)[0] + lo + pbase
                    if sel.size:
                        ys = np.asarray(res.results[c][f"ys{col}"])
                        r = (ivx * INTERVAL + k) * PATCHES
                        img_flat[TOK_PER_CORE * c + r + sel] = ys[sel]
    img = hs[0, IMG_START : IMG_START + IMG_LEN]
    outv = img_flat.reshape(NUM_FRAMES, PATCHES, HIDDEN)
    outv[:, 192:PATCHES, :] = _host_runt(img).reshape(NUM_FRAMES, 4, HIDDEN)
    return out


# revision 20
# speedup vs baseline: 1.0021x; 1.0021x over previous
"""CMC (Compressed Memory Compression) kernel for Trainium2 — 8 NeuronCores.

Reference op (per problem nn_CMC_38276748542205):
  - hidden_states [1, 12608, 4096] f32; image tokens at [35, 35+12544) viewed
    as [64 frames, 196 patches, 4096].
  - Frames form 16 intervals of 4; I-frame at position 3 of each interval.
  - SAD(token, I-frame token at same patch) over dim; mask = SAD < 1.12*4096.
  - Masked tokens replaced by the interval's I-frame token.

Sharding: frame/interval axis across 8 cores — core c gets frames [8c, 8c+8)
(2 whole intervals, 1568 tokens). Text tokens (64 rows) pass through on host.

Device kernel (per core, SPMD) — sparse-scatter formulation:
  The replacement value for a masked token is bit-exactly the interval's
  I-frame token, and unmasked tokens are bit-exactly the input — so a full
  write-back (25.7 MB/core) is wasted HBM traffic.  Instead:
  - stream patch-major chunks as [128 patches, 2 frames, 4096] half-tiles
    (I-half f2+f3, P-half f0+f1) via strided 4 MB DMAs on the two HWDGE
    rings;
  - DVE: d_k = p3 - p_k; ACT: |d_k| in-place with per-2048-chunk
    accumulation -> SAD (chunked so fp32 summation error stays below the
    min |SAD-thr| margin of ~0.034 — the 2e-2 rel-err budget only allows
    ~2 flipped tokens, so the SAD math must stay exact-fp32);
  - DVE: m = (sad < thr) per-partition 0/1 into a [128, 12] mask tile
    (one column per interval/frame/chunk), stored once at the end (6 KB)
    for the host merge;
  - scatter index: idx = iota_col*m + BIG -> masked rows get their patch
    row, unmasked rows get BIG (OOB); gpsimd indirect DMA with
    bounds_check silently skips OOB rows, writing ONLY the ~26% replaced
    rows (the f3 tile) into that (iv,k,chunk)'s own output tensor.
    Disjoint write sets per tensor keep the tile scheduler from chaining
    the scatters on a conservative WAW hazard; indirect DMA APs must
    start at partition 0 (non-zero start wedges the device), so chunk B
    scatters a [0:96] window with rows [0:32) forced OOB.
  Device traffic: ~24 MB loads + ~5 MB scatter vs 51.4 MB baseline. Host
  merge: out starts as a copy of the input; only mask-selected rows are
  copied from the scatter outputs (everything else is already correct by
  identity).

DMA shape rules (measured on HW):
  - the 16 SDMA engines split a transfer's partition dim into gcd(P,16)
    groups -> P must be a multiple of 16;
  - even SBUF AXI ports serve partitions <64, odd ports >=64 -> full rate
    needs the window balanced across the 64-boundary (128 rows, or 64
    rows at [32:96]; 64 rows at [0:64) run ~half rate);
  - compute APs must start at partition 0 (32/96 allow <=32 rows, 64
    allows <=64).
Patch coverage: chunk A = patches 0-127 at [0:128]; chunk B = patches
128-191 at partitions [32:96] (compute on [0:96]; the never-loaded rows
[0:32) produce garbage masks that are ignored: their scatter indices are
forced OOB and the host decodes only [32:96)). Patches 192-195 (the %16
runt) are handled host-side in numpy.
"""

import functools
import os

import numpy as np

# ---- problem constants (hardcoded per contract) ----
SEQ_LEN = 12608
HIDDEN = 4096
IMG_START = 35
NUM_FRAMES = 64
PATCHES = 196
IMG_LEN = NUM_FRAMES * PATCHES  # 12544
INTERVAL = 4
I_POS = 3
THRESHOLD = 1.12 * HIDDEN  # 4587.52

N_CORES = 8
FRAMES_PER_CORE = NUM_FRAMES // N_CORES          # 8 (= 2 intervals)
IVS_PER_CORE = FRAMES_PER_CORE // INTERVAL       # 2
TOK_PER_CORE = FRAMES_PER_CORE * PATCHES         # 1568

SAD_CHUNK = 2048       # accumulation chunk for SAD numerical accuracy
BIG = 4096.0           # OOB scatter index (> PATCHES-1 -> skipped)
N_MASK_COLS = IVS_PER_CORE * 3 * 2  # 12: (interval, P-frame k, chunk A/B)

# chunk table: (partition lo, partition hi, patch offset pbase) with
# patch = pbase + partition for partitions in [lo, hi)
CHUNKS = ((0, 128, 0), (32, 96, 96))


def _kernel_body(tc, ys_aps, ym_ap, x_ap):
    import concourse.bass as bass
    from concourse import mybir

    nc = tc.nc
    AF = mybir.ActivationFunctionType
    OP = mybir.AluOpType
    f32 = mybir.dt.float32
    i32 = mybir.dt.int32
    sim_init = bool(os.environ.get("CMC_SIM_INIT"))

    xv = x_ap.rearrange("(f p) d -> p f d", f=FRAMES_PER_CORE, p=PATCHES)

    import contextlib

    with contextlib.ExitStack() as ctx:
        pI_pool = ctx.enter_context(tc.tile_pool(name="pI", bufs=3))
        pP_pool = ctx.enter_context(tc.tile_pool(name="pP", bufs=2))
        d_pool = ctx.enter_context(tc.tile_pool(name="d", bufs=2))
        small_pool = ctx.enter_context(tc.tile_pool(name="small", bufs=12))
        hold_pool = ctx.enter_context(tc.tile_pool(name="hold", bufs=1))

        n_sad_chunks = HIDDEN // SAD_CHUNK

        # one-time tiles: per-chunk scatter index bases
        # iota_c[:, cb] = p + pbase - BIG
        iota_i = hold_pool.tile([128, 1], i32, tag="iotai")
        nc.gpsimd.iota(iota_i[:, :], [[0, 1]], base=0, channel_multiplier=1)
        iota_f = hold_pool.tile([128, 1], f32, tag="iotaf")
        nc.vector.tensor_copy(iota_f[:, :], iota_i[:, :])
        iota_c = hold_pool.tile([128, 2], f32, tag="iotac")
        for cb, (lo, hi, pbase) in enumerate(CHUNKS):
            nc.vector.tensor_scalar(
                iota_c[:, cb : cb + 1],
                iota_f[:, :],
                float(pbase - BIG),
                None,
                op0=OP.add,
            )
        mask_sb = hold_pool.tile([128, N_MASK_COLS], f32, tag="mask")
        nc.vector.memset(mask_sb[:, :], 0.0)

        def compute_k(pt3, ptk, iv, cb, k):
            lo, hi, pbase = CHUNKS[cb]
            q1 = hi
            d_t = d_pool.tile([128, HIDDEN], f32)
            nc.vector.tensor_tensor(
                d_t[:q1, :], pt3[:q1, :], ptk[:q1, :], op=OP.subtract
            )
            sadp = small_pool.tile([128, n_sad_chunks], f32, tag="sadp")
            for h in range(n_sad_chunks):
                # |d| in place (out aliases in); only accum_out is consumed
                nc.scalar.activation(
                    d_t[:q1, bass.ts(h, SAD_CHUNK)],
                    d_t[:q1, bass.ts(h, SAD_CHUNK)],
                    AF.Abs,
                    accum_out=sadp[:q1, h : h + 1],
                )
            col = iv * 6 + k * 2 + cb
            m_col = mask_sb[:, col : col + 1]
            # fused: m = (sadp0 + sadp1) < thr — both scalars per-partition
            nc.vector.tensor_scalar(
                m_col[:q1, :],
                sadp[:q1, 0:1],
                sadp[:q1, 1:2],
                float(THRESHOLD),
                op0=OP.add,
                op1=OP.is_lt,
            )
            # scatter index: masked -> patch row, unmasked -> BIG (OOB)
            idx_f = small_pool.tile([128, 1], f32, tag="idxf")
            nc.vector.tensor_scalar(
                idx_f[:q1, :],
                iota_c[:q1, cb : cb + 1],
                m_col[:q1, 0:1],
                BIG,
                op0=OP.mult,
                op1=OP.add,
            )
            idx_i = small_pool.tile([128, 1], i32, tag="idxi")
            nc.vector.tensor_copy(idx_i[:q1, :], idx_f[:q1, :])
            if lo > 0:
                # never-loaded rows: force out of bounds
                nc.vector.memset(idx_i[0:lo, :], int(BIG))
            # each (iv, k, chunk) scatter targets its own tensor: disjoint
            # write sets, so the tile scheduler doesn't chain them on a
            # conservative WAW hazard (one shared target serializes all 12
            # scatters on each other's DMA-completion semaphores)
            nc.gpsimd.indirect_dma_start(
                out=ys_aps[col],
                out_offset=bass.IndirectOffsetOnAxis(
                    ap=idx_i[0:hi, 0:1], axis=0
                ),
                in_=pt3[0:hi, :],
                in_offset=None,
                bounds_check=PATCHES - 1,
                oob_is_err=False,
            )

        ld = [0]

        def load(dst, src):
            # alternate the two HWDGE rings so streams interleave
            eng = nc.sync if ld[0] % 2 == 0 else nc.scalar
            ld[0] += 1
            eng.dma_start(dst, src)

        for iv in range(IVS_PER_CORE):
            f0 = iv * INTERVAL
            for cb, (lo, hi, pbase) in enumerate(CHUNKS):
                pw = slice(pbase + lo, pbase + hi)  # patch window in DRAM
                ptI = pI_pool.tile([128, 2, HIDDEN], f32, tag="ptI")
                ptP = pP_pool.tile([128, 2, HIDDEN], f32, tag="ptP")
                if sim_init and lo > 0:
                    nc.vector.memset(ptI[0:lo, :, :], 0.0)
                    nc.vector.memset(ptP[0:lo, :, :], 0.0)
                # 4 MB per dma_start: ring efficiency ~ bytes/(bytes +
                # ~2us fixed), so bigger transfers stream faster (2 MB
                # single-frame loads measured ~25% slower end to end)
                load(ptI[lo:hi, :, :], xv[pw, f0 + 2 : f0 + 4, :])
                load(ptP[lo:hi, :, :], xv[pw, f0 : f0 + 2, :])
                for k in (2, 0, 1):  # f=3 (I-frame) never changes
                    ptk = ptI[:, 0, :] if k == 2 else ptP[:, k, :]
                    compute_k(ptI[:, 1, :], ptk, iv, cb, k)

        # single 6 KB mask store for the host merge
        nc.sync.dma_start(ym_ap, mask_sb[:, :])


@functools.cache
def _build_nc():
    import concourse.bacc as bacc
    import concourse.tile as tile
    from concourse import mybir

    nc = bacc.Bacc(
        "TRN2",
        target_bir_lowering=False,
        debug=False,
        enable_asserts=False,
        num_devices=N_CORES,
    )
    x = nc.dram_tensor(
        "x", [TOK_PER_CORE, HIDDEN], mybir.dt.float32, kind="ExternalInput"
    ).ap()
    ys = [
        nc.dram_tensor(
            f"ys{c}", [PATCHES, HIDDEN], mybir.dt.float32, kind="ExternalOutput"
        ).ap()
        for c in range(N_MASK_COLS)
    ]
    ym = nc.dram_tensor(
        "ym", [128, N_MASK_COLS], mybir.dt.float32, kind="ExternalOutput"
    ).ap()
    with tile.TileContext(nc) as tc:
        _kernel_body(tc, ys, ym, x)
    nc.compile()
    return nc


def _in_maps(hs: np.ndarray):
    img = hs[0, IMG_START : IMG_START + IMG_LEN]
    maps = []
    for c in range(N_CORES):
        xc = img[TOK_PER_CORE * c : TOK_PER_CORE * (c + 1)]
        maps.append({"x": np.ascontiguousarray(xc)})
    return maps


def _host_runt(img: np.ndarray) -> np.ndarray:
    """Blend for patches 192-195 (the %16 runt the device skips): numpy."""
    iv = img.reshape(NUM_FRAMES // INTERVAL, INTERVAL, PATCHES, HIDDEN)
    runt = iv[:, :, 192:PATCHES, :]
    itok = runt[:, I_POS : I_POS + 1]
    d = itok.astype(np.float64) - runt.astype(np.float64)
    mask = np.abs(d).sum(-1) < THRESHOLD
    return np.where(mask[..., None], itok, runt).astype(np.float32)


def kernel(hidden_states: np.ndarray) -> np.ndarray:
    from concourse.bass_utils import run_bass_kernel_spmd

    hs = np.asarray(hidden_states, dtype=np.float32)
    assert hs.shape == (1, SEQ_LEN, HIDDEN), hs.shape
    nc = _build_nc()
    res = run_bass_kernel_spmd(nc, _in_maps(hs), list(range(N_CORES)))
    out = hs.copy()
    img_flat = out[0, IMG_START : IMG_START + IMG_LEN]  # [12544, 4096] view
    for c in range(N_CORES):
        ym = np.asarray(res.results[c]["ym"])   # [128, 12] 0/1 mask columns
        for ivx in range(IVS_PER_CORE):
            for k in range(3):
                for cb, (lo, hi, pbase) in enumerate(CHUNKS):
                    col = ivx * 6 + k * 2 + cb
                    sel = np.nonzero(ym[lo:hi, col] > 0.5)[0] + lo + pbase
                    if sel.size:
                        ys = np.asarray(res.results[c][f"ys{col}"])
                        r = (ivx * INTERVAL + k) * PATCHES
                        img_flat[TOK_PER_CORE * c + r + sel] = ys[sel]
    img = hs[0, IMG_START : IMG_START + IMG_LEN]
    outv = img_flat.reshape(NUM_FRAMES, PATCHES, HIDDEN)
    outv[:, 192:PATCHES, :] = _host_runt(img).reshape(NUM_FRAMES, 4, HIDDEN)
    return out
